# revision 17
# baseline (speedup 1.0000x reference)
"""DKT-PEBG kernel for Trainium2 (8 NeuronCores, batch-parallel).

Model: embedding lookup -> masked concat -> LSTM(128) -> per-token output
probability via gathered W_out rows.

Sharding: data-parallel over batch; core c handles rows [8c, 8c+8).

This version is hand-synchronized (no Tile auto-scheduling): every
instruction carries at most ONE inline semaphore wait (the hardware limit),
so no SEQ-blocking EventSemaphore instructions land on the recurrence
critical path. All other ordering is by per-engine program order plus
transitive semaphore coverage through the recurrence's own chain.

Cell math (all-Tanh form, single activation table; state D = 2c):
  s~ = tanh(preact/2) = 2*sigmoid(preact)-1 for gates o,i,f;  T = tanh(g)
  (g-gate weights are pre-scaled x2 on the host so tanh(preact/2)=tanh(g)).
  M  = (s~[i,f] + 1) * [T | D]   -> [2*sig_i*T | 2*sig_f*D]   (one DVE op)
  D' = M2*0.5 + M1 = 2*(sig_f*c + sig_i*tanh(g)) = 2c'        (one DVE op)
  tch = tanh(D/2) = tanh(c)
  h~ = (s~o + 1)*tch = 2h        (W_hh, W_out pre-scaled by 1/2 on host)

The y-mask on the embedding is folded into the gather: an extended
embedding table gets a zero row at index 10000 and the host computes
per-token gather indices (X or 10000).  Output sigmoid is also expressed
via tanh: prob = (tanh(z/2)+1)/2, folded into the final mask multiply.

Shapes (hardcoded): B=64, S=200, E=H=128, PRO_NUM=10000.
"""

from contextlib import ExitStack

import numpy as np

import concourse.bass as bass
import concourse.bacc as bacc
import concourse.mybir as mybir
from concourse.bass_utils import run_bass_kernel_spmd
from concourse.masks import make_identity

B, S = 64, 200
E = 128
H = 128
PRO_NUM = 10000
N_CORES = 8
BS = B // N_CORES              # 8 batch rows per core
NT = BS * S                    # 1600 tokens per core, token n = 8*s + b
NTILES = 13                    # ceil(1600/128); tile 12 has 64 valid tokens
NOUT = BS * (S - 1)            # 1592 output tokens
WB = H + 1                     # gathered W_out row + bias
F32 = mybir.dt.float32
BF16 = mybir.dt.bfloat16
I32 = mybir.dt.int32
AF = mybir.ActivationFunctionType
OP = mybir.AluOpType

# psr gate-column order [o, i, f, g] <- W row blocks (PyTorch i,f,g,o)
_GATE_SRC = (3, 0, 1, 2)
_SC_X = (1.0, 1.0, 1.0, 2.0)     # g-gate preact x2 (tanh-scale trick)
_SC_H = (0.5, 0.5, 0.5, 1.0)     # /2 for h~=2h, g-gate x2

# input-GEMM chunks: (base_token, n_tokens); chunk 0 small so the
# recurrence starts early
_CHUNKS = ((0, 64), (64, 320), (384, 256), (640, 512), (1152, 448))

RACY_NO_SELF_SEM = False       # skip DVE self-sem between M and D update


def _tok_w(t):
    return 128 if t < NTILES - 1 else NT - 128 * (NTILES - 1)


def _out_w(t):
    return 128 if t < NTILES - 1 else NOUT - 128 * (NTILES - 1)


def build_kernel():
    nc = bacc.Bacc("TRN2", target_bir_lowering=False, debug=False,
                   num_devices=N_CORES)

    # ---- DRAM I/O ----
    idx_d = nc.dram_tensor("idx", [128, 3 * NTILES], I32, kind="ExternalInput")
    mz_d = nc.dram_tensor("mz", [128, NTILES], F32, kind="ExternalInput")
    emb_d = nc.dram_tensor("emb", [PRO_NUM + 1, E], BF16, kind="ExternalInput")
    wgt_d = nc.dram_tensor("wgt", [128, 1536], BF16, kind="ExternalInput")
    bs_d = nc.dram_tensor("bsum", [128, 4], F32, kind="ExternalInput")
    wb_d = nc.dram_tensor("wb", [PRO_NUM, WB], F32, kind="ExternalInput")
    prob_d = nc.dram_tensor("prob", [128, NTILES], F32, kind="ExternalOutput")

    es = ExitStack()

    def sb(name, shape, dtype):
        return es.enter_context(nc.sbuf_tensor(name, shape, dtype))

    ident = sb("identsb", [128, 128], BF16)
    idx = sb("idxsb", [128, 3 * NTILES], I32)
    mz = sb("mzsb", [128, NTILES], F32)
    wgt = sb("wgtsb", [128, 1536], BF16)
    bsum_t = sb("bssb", [128, 4], F32)
    exa = sb("exasb", [128, NTILES * E], BF16)
    exb = sb("exbsb", [128, NTILES * E], BF16)
    xaT = sb("xaTsb", [128, NT], BF16)
    xbT = sb("xbTsb", [128, NT], BF16)
    xgb = sb("xgbsb", [128, 32 * S], BF16)
    hseq = sb("hseqsb", [128, 8 * S], BF16)
    sigD = sb("sigDsb", [128, 40], F32)     # [s~o|s~i|s~f|T|D]
    Mt = sb("Mtsb", [128, 16], F32)
    tch = sb("tchsb", [128, 8], F32)
    wgb = sb("wgbsb", [128, NTILES * WB], F32)
    hw = sb("hwsb", [128, 256], F32)        # out_tile scratch (2x parity)
    dcol = sb("dcolsb", [128, 4], F32)      # out_tile dot + tanh (2x parity)
    fence = sb("fencesb", [1, 1], F32)
    prob_sb = sb("probsb", [128, NTILES], F32)

    psr = es.enter_context(nc.psum_tensor("psrps", [128, 128], F32))   # 4x32
    pst = es.enter_context(nc.psum_tensor("pstps", [128, 512], BF16))  # 4x128
    psgs = [es.enter_context(nc.psum_tensor(f"psg{i}ps", [128, 512], F32))
            for i in range(4)]
    pho = es.enter_context(nc.psum_tensor("phops", [128, 256], BF16))  # 2x128

    semL = nc.alloc_semaphore("semL")
    semG = nc.alloc_semaphore("semG")
    semP = nc.alloc_semaphore("semP")
    semA = nc.alloc_semaphore("semA")
    semD = nc.alloc_semaphore("semD")
    semI = nc.alloc_semaphore("semI")
    my_sems = [semL, semG, semP, semA, semD, semI]
    cnt = {"L": 0, "G": 0, "P": 0, "A": 0, "D": 0}

    # device sems persist across model loads: reset DMA state bound to our
    # sems and zero them before any engine consumes them
    for sm in my_sems:
        nc.gpsimd.sem_clear(sm)
    nc.all_engine_barrier()

    def inc(instr, s, sem, dma=False):
        # DMA-engine sem updates must be multiples of 16 on TRN2; count
        # DMA-completion sems in units of 16.
        n = 16 if dma else 1
        bass.BassInstruction(instr.ins).then_inc(sem, n)
        cnt[s] += n
        return cnt[s]

    def wge(instr, sem, val):
        bass.BassInstruction(instr.ins).wait_op(sem, val, "sem-ge")

    # ---------------- loads ----------------
    inc(nc.sync.dma_start(idx.ap(), idx_d[:]), "L", semL, dma=True)
    n_idx_loads = cnt["L"]
    inc(nc.sync.dma_start(bsum_t.ap(), bs_d[:]), "L", semL, dma=True)
    inc(nc.sync.dma_start(mz.ap(), mz_d[:]), "L", semL, dma=True)
    inc(nc.sync.dma_start(wgt.ap(), wgt_d[:]), "L", semL, dma=True)
    n_loads = cnt["L"]
    ia = idx.ap()[:, 0:NTILES]
    ib = idx.ap()[:, NTILES:2 * NTILES]
    iw = idx.ap()[:, 2 * NTILES:3 * NTILES]
    wx = wgt.ap()[:, 0:1024]
    whh = wgt.ap()[:, 1024:1536]
    bsum = bsum_t.ap()

    # ---------------- init (gpsimd) ----------------
    make_identity(nc, ident.ap())
    nc.gpsimd.memset(sigD.ap()[:, 32:40], 0.0)      # D = 0
    nc.gpsimd.memset(prob_sb.ap(), 0.0)
    nc.gpsimd.engine_nop().then_inc(semI, 1)

    # ---------------- gathers (gpsimd SWDGE) ----------------
    g_ex = {}
    for t in range(NTILES):
        w = _tok_w(t)
        i = nc.gpsimd.indirect_dma_start(
            out=exa.ap()[0:w, E * t:E * t + E], out_offset=None, in_=emb_d[:],
            in_offset=bass.IndirectOffsetOnAxis(ap=ia[0:w, t:t + 1],
                                                axis=0))
        wge(i, semL, n_idx_loads)
        g_ex[("a", t)] = inc(i, "G", semG, dma=True)
        i = nc.gpsimd.indirect_dma_start(
            out=exb.ap()[0:w, E * t:E * t + E], out_offset=None, in_=emb_d[:],
            in_offset=bass.IndirectOffsetOnAxis(ap=ib[0:w, t:t + 1],
                                                axis=0))
        g_ex[("b", t)] = inc(i, "G", semG, dma=True)
    for t in range(NTILES):
        w = _out_w(t)
        i = nc.gpsimd.indirect_dma_start(
            out=wgb.ap()[0:w, WB * t:WB * (t + 1)], out_offset=None,
            in_=wb_d[:],
            in_offset=bass.IndirectOffsetOnAxis(ap=iw[0:w, t:t + 1],
                                                axis=0))
        inc(i, "G", semG, dma=True)
    n_gathers = cnt["G"]

    # ---------------- warm-up ----------------
    for k in range(10):
        i = nc.tensor.matmul(psr.ap()[:, 96:128], ident.ap(),
                             ident.ap()[:, 0:32], start=True, stop=True)
        if k == 0:
            wge(i, semI, 1)
        elif k == 9:
            wge(i, semL, n_loads)
        inc(i, "P", semP)
    i = nc.scalar.activation(tch.ap(), psr.ap()[:, 96:104], AF.Tanh)
    wge(i, semP, cnt["P"])

    # ---------------- building blocks ----------------
    tr_done = {}     # ('a'|'b', t) -> P count after transpose
    cp_done = {}     # ('a'|'b', t) -> D count after copy

    def emit_tr(t):
        """transpose gathered tile t (a and b) into xaT/xbT via PSUM."""
        w = _tok_w(t)
        for half, src, dstT in (("a", exa, xaT), ("b", exb, xbT)):
            slot = pst.ap()[:, 128 * ((2 * t + (half == "b")) % 4):]
            slot = slot[:, 0:128]
            i = nc.tensor.transpose(slot[:, 0:w],
                                    src.ap()[0:w, E * t:E * t + E],
                                    ident.ap()[0:w, 0:w])
            wge(i, semG, g_ex[(half, t)])
            tr_done[(half, t)] = inc(i, "P", semP)
            i = nc.vector.tensor_copy(dstT.ap()[:, 128 * t:128 * t + w],
                                      slot[:, 0:w])
            wge(i, semP, tr_done[(half, t)])
            cp_done[(half, t)] = inc(i, "D", semD)

    last_bias = {}   # (chunk, j) -> D count after its bias-add

    def emit_gemm(c, j):
        """input-GEMM pass j (gate block) for chunk c, then bias-add."""
        base, w = _CHUNKS[c]
        dep = cp_done[("b", (base + w - 1) // 128)]
        if c == 0:
            bank = psgs[0]
            off = w * j
        else:
            bank = psgs[j]
            off = 0
            if c == 1 and j == 0:
                dep = max(dep, last_bias[(0, 3)])
            elif c >= 2:
                dep = max(dep, last_bias[(c - 1, j)])
        i = nc.tensor.matmul(bank.ap()[:, off:off + w],
                             wx[:, 128 * j:128 * (j + 1)],
                             xaT.ap()[:, base:base + w],
                             start=True, stop=False)
        wge(i, semD, dep)
        inc(i, "P", semP)
        i = nc.tensor.matmul(bank.ap()[:, off:off + w],
                             wx[:, 512 + 128 * j:512 + 128 * (j + 1)],
                             xbT.ap()[:, base:base + w],
                             start=False, stop=True)
        pcount = inc(i, "P", semP)
        dst = xgb.ap()[:, 4 * base: 4 * base + 32 * (w // 8)] \
            .rearrange("p (q x) -> p q x", x=32)[:, :, 8 * j:8 * j + 8]
        src = bank.ap()[:, off:off + w].rearrange("p (q x) -> p q x", x=8)
        i = nc.vector.tensor_scalar(dst, src, bsum[:, j:j + 1], None,
                                    op0=OP.add)
        wge(i, semP, pcount)
        last_bias[(c, j)] = inc(i, "D", semD)

    h_done = [0] * S     # D count after h~(t)
    outstt_done = {}
    outact_done = {}

    def emit_out1(t):
        """out_tile part 1: transpose h block + dot with gathered W_out."""
        w = _out_w(t)
        slot = pho.ap()[:, 128 * (t % 2):128 * (t % 2) + 128]
        i = nc.tensor.transpose(slot[0:w, :],
                                hseq.ap()[:, 128 * t:128 * t + w],
                                ident.ap())
        wge(i, semD, h_done[16 * t + (w - 1) // 8])
        pc = inc(i, "P", semP)
        par = t % 2
        i = nc.vector.scalar_tensor_tensor(
            out=hw.ap()[0:w, 128 * par:128 * par + 128], in0=slot[0:w, :],
            scalar=1.0, in1=wgb.ap()[0:w, WB * t:WB * t + H],
            op0=OP.mult, op1=OP.mult,
            accum_out=dcol.ap()[0:w, 2 * par:2 * par + 1])
        wge(i, semP, pc)
        outstt_done[t] = inc(i, "D", semD)

    def emit_out2(t):
        """out_tile part 2: prob = (tanh((d+b)/2)+1) * mz   (mz = mask/2)."""
        w = _out_w(t)
        par = t % 2
        i = nc.scalar.activation(
            dcol.ap()[0:w, 2 * par + 1:2 * par + 2],
            dcol.ap()[0:w, 2 * par:2 * par + 1], AF.Tanh,
            bias=wgb.ap()[0:w, WB * t + H:WB * (t + 1)], scale=0.5)
        wge(i, semD, outstt_done[t])
        ac = inc(i, "A", semA)
        i = nc.vector.scalar_tensor_tensor(
            out=prob_sb.ap()[0:w, t:t + 1],
            in0=dcol.ap()[0:w, 2 * par + 1:2 * par + 2],
            scalar=1.0, in1=mz.ap()[0:w, t:t + 1], op0=OP.add, op1=OP.mult)
        wge(i, semA, ac)
        outact_done[t] = inc(i, "D", semD)

    def emit_gnop():
        # one-time DVE fence: all gathers (incl. W_out rows) are done
        i = nc.vector.tensor_copy(fence.ap(), prob_sb.ap()[0:1, 0:1])
        wge(i, semG, n_gathers)
        inc(i, "D", semD)

    def emit_probdma_main():
        i = nc.sync.dma_start(prob_d[:][:, 0:12], prob_sb.ap()[:, 0:12])
        wge(i, semD, outact_done[11])
        inc(i, "L", semL, dma=True)

    # ---------------- startup: tile 0 + GEMM chunk 0 ----------------
    emit_tr(0)
    for j in range(4):
        emit_gemm(0, j)

    # ---------------- side-work schedule ----------------
    # one tile's transpose+copy and at most one GEMM j-pass per step keeps
    # side DVE work inside the post-h~ idle window (~1us)
    side = {}
    _TR_STEP = {1: 1, 2: 2, 3: 3, 4: 4, 5: 5, 6: 6, 7: 7, 8: 8,
                9: 11, 10: 12, 11: 14, 12: 15}
    for t, st in _TR_STEP.items():
        side.setdefault(st, []).append(("tr", t))
    for j in range(4):
        side.setdefault(3 + j, []).append(("gemm", 1, j))
        side.setdefault(8 + j, []).append(("gemm", 2, j))
        side.setdefault(12 + j, []).append(("gemm", 3, j))
        side.setdefault(16 + j, []).append(("gemm", 4, j))
    side.setdefault(38, []).append(("gnop",))
    for t in range(12):
        st = max(16 * t + 17, 40 + 13 * t)
        side.setdefault(st, []).append(("out1", t))
        side.setdefault(st + 1, []).append(("out2", t))
    side.setdefault(196, []).append(("probdma",))

    # ---------------- recurrence ----------------
    Dst = sigD.ap()[:, 32:40]
    sg = sigD.ap()

    for t in range(S):
        pb = psr.ap()[:, 32 * (t % 4):32 * (t % 4) + 32]
        if t > 0:
            hprev = hseq.ap()[:, 8 * (t - 1):8 * t]
            for j in range(4):
                i = nc.tensor.matmul(pb[:, 8 * j:8 * j + 8],
                                     whh[:, 128 * j:128 * (j + 1)],
                                     hprev, start=(j == 0), stop=False)
                if j == 0:
                    wge(i, semD, h_done[t - 1])
                    inc(i, "P", semP)
        i = nc.tensor.matmul(pb, ident.ap(), xgb.ap()[:, 32 * t:32 * t + 32],
                             start=(t == 0), stop=True)
        if t == 0:
            wge(i, semD, last_bias[(0, 3)])
        pcount = inc(i, "P", semP)

        i = nc.scalar.activation(sg[:, 0:32], pb, AF.Tanh, scale=0.5)
        wge(i, semP, pcount)
        acount = inc(i, "A", semA)

        i = nc.vector.scalar_tensor_tensor(
            out=Mt.ap(), in0=sg[:, 8:24], scalar=1.0, in1=sg[:, 24:40],
            op0=OP.add, op1=OP.mult)
        wge(i, semA, acount)
        mcount = inc(i, "D", semD)

        i = nc.vector.scalar_tensor_tensor(
            out=Dst, in0=Mt.ap()[:, 8:16], scalar=0.5, in1=Mt.ap()[:, 0:8],
            op0=OP.mult, op1=OP.add)
        if not RACY_NO_SELF_SEM:
            wge(i, semD, mcount)
        dcount = inc(i, "D", semD)

        i = nc.scalar.activation(tch.ap(), Dst, AF.Tanh, scale=0.5)
        wge(i, semD, dcount)
        acount = inc(i, "A", semA)

        i = nc.vector.scalar_tensor_tensor(
            out=hseq.ap()[:, 8 * t:8 * t + 8], in0=sg[:, 0:8], scalar=1.0,
            in1=tch.ap(), op0=OP.add, op1=OP.mult)
        wge(i, semA, acount)
        h_done[t] = inc(i, "D", semD)

        for item in side.get(t, ()):
            if item[0] == "tr":
                emit_tr(item[1])
            elif item[0] == "gemm":
                emit_gemm(item[1], item[2])
            elif item[0] == "out1":
                emit_out1(item[1])
            elif item[0] == "out2":
                emit_out2(item[1])
            elif item[0] == "gnop":
                emit_gnop()
            elif item[0] == "probdma":
                emit_probdma_main()

    # ---------------- tail ----------------
    emit_out1(12)
    emit_out2(12)
    with nc.allow_non_contiguous_dma(reason="tiny 1-col tail store"):
        i = nc.sync.dma_start(prob_d[:][:, 12:13], prob_sb.ap()[:, 12:13])
    wge(i, semD, outact_done[12])
    inc(i, "L", semL, dma=True)

    # leave sems cleared for the next model load (framework convention)
    nc.all_engine_barrier()
    for sm in my_sems:
        nc.gpsimd.sem_clear(sm)
    nc.all_engine_barrier()

    nc.compile()
    return nc


_CACHED = None


def _get_kernel():
    global _CACHED
    if _CACHED is None:
        _CACHED = build_kernel()
    return _CACHED


def _prep_shared(pro_embed, W_ih, W_hh, b_ih, b_hh, W_out, b_out):
    wx_h = np.empty((128, 1024), np.float32)
    whh_h = np.empty((128, 512), np.float32)
    bias_h = np.empty((128, 4), np.float32)
    for j, g in enumerate(_GATE_SRC):
        blk = slice(g * 128, (g + 1) * 128)
        wx_h[:, j * 128:(j + 1) * 128] = _SC_X[j] * W_ih[blk, 0:128].T
        wx_h[:, 512 + j * 128:512 + (j + 1) * 128] = \
            _SC_X[j] * W_ih[blk, 128:256].T
        whh_h[:, j * 128:(j + 1) * 128] = _SC_H[j] * W_hh[blk, :].T
        bias_h[:, j] = _SC_X[j] * (b_ih[blk] + b_hh[blk])
    emb_h = np.zeros((PRO_NUM + 1, E), np.float32)
    emb_h[:PRO_NUM] = pro_embed
    wb_h = np.empty((PRO_NUM, WB), np.float32)
    wb_h[:, :H] = 0.5 * W_out
    wb_h[:, H] = 0.5 * b_out
    import ml_dtypes
    bf16 = ml_dtypes.bfloat16
    wgt_h = np.concatenate([wx_h, whh_h], axis=1)
    return dict(
        emb=emb_h.astype(bf16),
        wgt=np.ascontiguousarray(wgt_h).astype(bf16),
        bsum=np.ascontiguousarray(bias_h),
        wb=wb_h,
    )


def _tile_layout(flat, pad_val, dtype):
    out = np.full(NTILES * 128, pad_val, dtype)
    out[:len(flat)] = flat
    return np.ascontiguousarray(out.reshape(NTILES, 128).T)


def kernel(X, y, pro_embed, W_ih, W_hh, b_ih, b_hh, W_out, b_out,
           _trace=False, **_):
    X = np.asarray(X, np.int64)
    y = np.asarray(y, np.int64)
    shared = _prep_shared(np.asarray(pro_embed, np.float32),
                          np.asarray(W_ih, np.float32),
                          np.asarray(W_hh, np.float32),
                          np.asarray(b_ih, np.float32),
                          np.asarray(b_hh, np.float32),
                          np.asarray(W_out, np.float32),
                          np.asarray(b_out, np.float32))
    in_maps = []
    for c in range(N_CORES):
        rows = slice(c * BS, (c + 1) * BS)
        xt = X[rows].T.reshape(-1)          # token n = 8*s + b
        yt = y[rows].T.reshape(-1)
        ia_f = np.where(yt == 0, xt, PRO_NUM).astype(np.int32)
        ib_f = np.where(yt == 1, xt, PRO_NUM).astype(np.int32)
        xs = xt[8:]                          # X shifted one step
        iw_f = np.maximum(xs - 1, 0).astype(np.int32)
        mz_f = np.where(xs != 0, 0.5, 0.0).astype(np.float32)
        idx_h = np.concatenate([_tile_layout(ia_f, PRO_NUM, np.int32),
                                _tile_layout(ib_f, PRO_NUM, np.int32),
                                _tile_layout(iw_f, 0, np.int32)], axis=1)
        in_maps.append(dict(
            idx=np.ascontiguousarray(idx_h),
            mz=_tile_layout(mz_f, 0.0, np.float32),
            **shared))

    nc = _get_kernel()
    res = run_bass_kernel_spmd(nc, in_maps, core_ids=list(range(N_CORES)),
                               trace=_trace)
    out = np.empty((B, S - 1), np.float32)
    for c in range(N_CORES):
        arr = res.results[c]["prob"]          # [128, 13], n = 128*t + p
        flat = arr.T.reshape(-1)[:NOUT].reshape(S - 1, BS)
        out[c * BS:(c + 1) * BS, :] = flat.T
    if _trace:
        return out, res
    return out


# revision 21
# speedup vs baseline: 1.0721x; 1.0721x over previous
"""DKT-PEBG kernel for Trainium2 (8 NeuronCores, batch-parallel).

Model: embedding lookup -> masked concat -> LSTM(128) -> per-token output
probability via gathered W_out rows (avoids materializing [B,S,10000]).

Sharding: data-parallel over batch. Core c handles batch rows [8c, 8c+8).
No collectives; host splits inputs / concatenates outputs.

Shapes (hardcoded): B=64, S=200, E=H=128, PRO_NUM=10000.

Recurrence trick: gate order [i,f,o,g] with the g-gate pre-activation
prescaled by 2 on the host, so one Sigmoid over all 4 gates suffices:
tanh(g) = 2*sigmoid(2g) - 1. Input-GEMM chunks and gathers are emitted
interleaved with the first recurrence steps so the scheduler pipelines
them instead of serializing ~45us of startup.
"""

import numpy as np

import concourse.bass as bass
import concourse.bacc as bacc
import concourse.mybir as mybir
import concourse.tile as tile
from concourse.bass_utils import run_bass_kernel_spmd
from concourse.masks import make_identity

B, S = 64, 200
E = 128
H = 128
PRO_NUM = 10000
N_CORES = 8
BS = B // N_CORES              # 8 batch rows per core
NT = BS * S                    # 1600 tokens per core, token n = 8*s + b
NTILES = 13                    # ceil(1600/128); tile 12 has 64 valid tokens
NOUT = BS * (S - 1)            # 1592 output tokens
WB = H + 1                     # gathered W_out row + bias
F32 = mybir.dt.float32
I32 = mybir.dt.int32

_GATE_SRC = (3, 0, 1, 2)       # col blocks [o, i, f, g] <- W_ih row blocks (i,f,g,o)

# input-GEMM chunks in tiles: (first_tile, n_tiles); chunk 0 small so the
# recurrence can start early
_CHUNKS = ((0, 1), (1, 2), (3, 2), (5, 4), (9, 4))


def _tok_w(t):
    return 128 if t < NTILES - 1 else NT - 128 * (NTILES - 1)


def _out_w(t):
    return 128 if t < NTILES - 1 else NOUT - 128 * (NTILES - 1)


def build_kernel():
    nc = bacc.Bacc("TRN2", target_bir_lowering=False, debug=False,
                   num_devices=N_CORES)

    # ---- I/O ----
    xt = nc.dram_tensor("xt", [209, BS], I32, kind="ExternalInput")   # X.T slice, padded
    yt = nc.dram_tensor("yt", [208, BS], I32, kind="ExternalInput")   # y.T slice, padded
    emb = nc.dram_tensor("emb", [PRO_NUM, E], F32, kind="ExternalInput")
    wx = nc.dram_tensor("wx", [128, 1024], F32, kind="ExternalInput")  # W_ih.T blocks [A|B]
    whh = nc.dram_tensor("whh", [128, 512], F32, kind="ExternalInput")  # W_hh.T blocks
    bsum = nc.dram_tensor("bsum", [128, 4], F32, kind="ExternalInput")  # b_ih+b_hh blocks
    wb = nc.dram_tensor("wb", [PRO_NUM, WB], F32, kind="ExternalInput")  # [W_out | b_out]
    prob = nc.dram_tensor("prob", [NTILES * 128], F32, kind="ExternalOutput")

    AF = mybir.ActivationFunctionType
    OP = mybir.AluOpType

    with tile.TileContext(nc) as tc:
        with (
            tc.tile_pool(name="persist", bufs=1) as pp,
            tc.tile_pool(name="work", bufs=4) as wp,
            tc.tile_pool(name="exp", bufs=13) as expool,
            tc.tile_pool(name="rec", bufs=4) as rp,
            tc.tile_pool(name="ps_tr", bufs=2, space="PSUM") as ps_tr,
            tc.tile_pool(name="ps_mm", bufs=3, space="PSUM") as ps_mm,
            tc.tile_pool(name="ps_rec", bufs=3, space="PSUM") as ps_rec,
        ):
            # ---- persistent SBUF ----
            ident = pp.tile([128, 128], F32, tag="ident")
            wx_sb = pp.tile([128, 1024], F32, tag="wx_sb")
            whh_sb = pp.tile([128, 512], F32, tag="whh_sb")
            bias_sb = pp.tile([128, 4], F32, tag="bias_sb")
            ix_all = pp.tile([128, NTILES], I32, tag="ix_all")
            ixs_all = pp.tile([128, NTILES], I32, tag="ixs_all")
            y_all = pp.tile([128, NTILES], I32, tag="y_all")
            y_f = pp.tile([128, NTILES], F32, tag="y_f")
            m1 = pp.tile([128, NTILES], F32, tag="m1")
            m2 = pp.tile([128, NTILES], F32, tag="m2")
            ixm1 = pp.tile([128, NTILES], I32, tag="ixm1")
            ixs_f = pp.tile([128, NTILES], F32, tag="ixs_f")
            mnz = pp.tile([128, NTILES], F32, tag="mnz")
            xaT = pp.tile([128, NT], F32, tag="xaT")
            xbT = pp.tile([128, NT], F32, tag="xbT")
            xgb = pp.tile([128, 32 * S], F32, tag="xgb")
            hseq = pp.tile([128, NT], F32, tag="hseq")
            wgb_all = pp.tile([128, NTILES * WB], F32, tag="wgb_all")
            sigD = pp.tile([128, 40], F32, tag="sigD")
            Mt = pp.tile([128, 16], F32, tag="Mt")
            prob_sb = pp.tile([128, NTILES], F32, tag="prob_sb")

            make_identity(nc, ident[:])
            nc.gpsimd.memset(prob_sb[:], 0.0)
            nc.gpsimd.memset(sigD[:, 32:40], 0.0)

            # ---- loads ----
            xt_flat = xt[:].rearrange("s b -> (s b)")
            yt_flat = yt[:].rearrange("s b -> (s b)")
            nc.sync.dma_start(
                ix_all[:], xt_flat[0:1664].rearrange("(t p) -> p t", p=128))
            nc.sync.dma_start(
                y_all[:], yt_flat[0:1664].rearrange("(t p) -> p t", p=128))
            nc.sync.dma_start(
                ixs_all[:], xt_flat[8:1672].rearrange("(t p) -> p t", p=128))
            nc.sync.dma_start(wx_sb[:], wx[:])
            nc.sync.dma_start(whh_sb[:], whh[:])
            nc.sync.dma_start(bias_sb[:], bsum[:])

            # warm the ACT sigmoid/tanh table set off the critical path
            warm = wp.tile([1, 1], F32, tag="warm")
            nc.scalar.activation(warm[:], ident[0:1, 0:1], AF.Tanh)

            # warm the PE HAM clock gate so the startup GEMM runs at full rate
            for _ in range(10):
                pwm = ps_mm.tile([128, 512], F32, tag="psg")
                nc.tensor.matmul(pwm[:, 0:128], ident[:], ident[:],
                                 start=True, stop=True)

            # masks: m1 = (y==0), m2 = (y==1); padding y==-1 -> 0,0
            nc.vector.tensor_copy(y_f[:], y_all[:])
            nc.vector.tensor_scalar(m1[:], y_f[:], 0.0, None, op0=OP.is_equal)
            nc.vector.tensor_scalar(m2[:], y_f[:], 1.0, None, op0=OP.is_equal)

            ex_tiles = {}

            def gather_tile(t):
                w = _tok_w(t)
                ex_t = expool.tile([128, E], F32, tag="ex")
                ex_tiles[t] = ex_t
                nc.gpsimd.indirect_dma_start(
                    out=ex_t[0:w, :], out_offset=None, in_=emb[:],
                    in_offset=bass.IndirectOffsetOnAxis(
                        ap=ix_all[0:w, t:t + 1], axis=0))

            def finish_tile(t):
                """mask + transpose gathered tile t into xaT/xbT"""
                w = _tok_w(t)
                ex_t = ex_tiles.pop(t)
                xa_t = wp.tile([128, E], F32, tag="xa")
                xb_t = wp.tile([128, E], F32, tag="xb")
                nc.vector.tensor_scalar(xa_t[0:w, :], ex_t[0:w, :],
                                        m1[0:w, t:t + 1], None, op0=OP.mult)
                nc.vector.tensor_scalar(xb_t[0:w, :], ex_t[0:w, :],
                                        m2[0:w, t:t + 1], None, op0=OP.mult)
                psa = ps_tr.tile([128, 128], F32, tag="psa")
                nc.tensor.transpose(psa[:, 0:w], xa_t[0:w, :], ident[0:w, 0:w])
                nc.vector.tensor_copy(xaT[:, 128 * t:128 * t + w], psa[:, 0:w])
                psb = ps_tr.tile([128, 128], F32, tag="psa")
                nc.tensor.transpose(psb[:, 0:w], xb_t[0:w, :], ident[0:w, 0:w])
                nc.vector.tensor_copy(xbT[:, 128 * t:128 * t + w], psb[:, 0:w])

            pending_psg = {}

            def process_tile(t):
                gather_tile(t)
                finish_tile(t)

            def gemm_a(base, w, j):
                psg = ps_mm.tile([128, 512], F32, tag="psg")
                pending_psg[(base, j)] = psg
                nc.tensor.matmul(
                    psg[:, 0:w], wx_sb[:, 128 * j:128 * (j + 1)],
                    xaT[:, base:base + w], start=True, stop=False)

            def gemm_b(base, w, j):
                psg = pending_psg.pop((base, j))
                nc.tensor.matmul(
                    psg[:, 0:w], wx_sb[:, 512 + 128 * j:512 + 128 * (j + 1)],
                    xbT[:, base:base + w], start=False, stop=True)
                dst = xgb[:, 4 * base: 4 * base + 32 * (w // 8)] \
                    .rearrange("p (q x) -> p q x", x=32)[:, :, 8 * j:8 * j + 8]
                src = psg[:, 0:w].rearrange("p (q x) -> p q x", x=8)
                nc.vector.tensor_scalar(dst, src, bias_sb[:, j:j + 1], None,
                                        op0=OP.add)

            def gemm_range(base, w, j):
                gemm_a(base, w, j)
                gemm_b(base, w, j)

            def gather_wb(t):
                w = _out_w(t)
                nc.gpsimd.indirect_dma_start(
                    out=wgb_all[0:w, WB * t:WB * (t + 1)], out_offset=None,
                    in_=wb[:],
                    in_offset=bass.IndirectOffsetOnAxis(
                        ap=ixm1[0:w, t:t + 1], axis=0))

            def out_tile(t):
                '''prob = sigmoid(h . W_out[idx] + b_out[idx]) * (X != 0)'''
                w = _out_w(t)
                pst = ps_tr.tile([128, 128], F32, tag="psa")
                nc.tensor.transpose(pst[0:w, :], hseq[:, 128 * t:128 * t + w],
                                    ident[:])
                hw_t = wp.tile([128, 128], F32, tag="hw")
                d_t = wp.tile([128, 1], F32, tag="d")
                nc.vector.scalar_tensor_tensor(
                    out=hw_t[0:w, :], in0=pst[0:w, :], scalar=1.0,
                    in1=wgb_all[0:w, WB * t:WB * t + H],
                    op0=OP.mult, op1=OP.mult, accum_out=d_t[0:w, :])
                p_t = wp.tile([128, 1], F32, tag="p")
                nc.scalar.activation(p_t[0:w, :], d_t[0:w, :], AF.Tanh,
                                     bias=wgb_all[0:w, WB * t + H:WB * (t + 1)],
                                     scale=0.5)
                nc.vector.scalar_tensor_tensor(
                    out=prob_sb[0:w, t:t + 1], in0=p_t[0:w, :], scalar=1.0,
                    in1=mnz[0:w, t:t + 1], op0=OP.add, op1=OP.mult)

            # interleave schedule: step index -> list of thunks.
            # chunk c tokens start at step 16*_CHUNKS[c][0]; stay ahead of it.
            side = {}
            tile_steps = {1: (1, 2), 2: (11, 13), 3: (30, 34, 38, 42),
                          4: (60, 64, 68, 72)}
            gemm_steps = {1: 3, 2: 16, 3: 45, 4: 76}
            for j in range(4):                  # second half of tile 0
                side.setdefault(2 + j, []).append(("gemm0b", j))
            for c in (1, 2, 3, 4):
                t0, ntl = _CHUNKS[c]
                for k in range(ntl):
                    side.setdefault(tile_steps[c][k], []).append(
                        ("tile", t0 + k))
                for j in range(4):
                    s0 = gemm_steps[c] + 2 * j
                    side.setdefault(s0, []).append(("gemm_a", c, j))
                    side.setdefault(s0 + 1, []).append(("gemm_b", c, j))
            late_out = []
            for t in range(NTILES):             # output tiles once h is ready
                # tile t reads h(s) up to s = 16t + (w-1)//8
                smax = 16 * t + (_out_w(t) - 1) // 8
                step = max(smax + 1, 140 + 4 * t)
                if step <= S - 1:
                    side.setdefault(step, []).append(("out", t))
                else:
                    late_out.append(t)
            side.setdefault(196, []).append(("probdma",))

            # ---- chunk 0 (first 64 tokens) then the recurrence ----
            process_tile(0)
            for j in range(4):
                gemm_range(0, 64, j)

            # index prep for the W_out gathers (off the sigma(0) path)
            nc.vector.tensor_scalar(ixm1[:], ixs_all[:], 1, 0,
                                    op0=OP.subtract, op1=OP.max)
            nc.vector.tensor_copy(ixs_f[:], ixs_all[:])
            nc.vector.tensor_scalar(mnz[:], ixs_f[:], 0.0, 0.5,
                                    op0=OP.not_equal, op1=OP.mult)

            # queue every remaining gather now; the Pool engine drains them
            # in the background while the recurrence runs on PE/ACT/DVE
            for t in range(1, NTILES):
                gather_tile(t)
            for t in range(NTILES):
                gather_wb(t)

            for t in range(S):
                psr = ps_rec.tile([128, 32], F32, tag="psr")
                nc.tensor.matmul(psr[:], ident[:], xgb[:, 32 * t:32 * t + 32],
                                 start=True, stop=(t == 0))
                if t > 0:
                    hprev = hseq[:, 8 * (t - 1):8 * t]
                    for j in range(4):
                        nc.tensor.matmul(
                            psr[:, 8 * j:8 * j + 8],
                            whh_sb[:, 128 * j:128 * (j + 1)], hprev,
                            start=False, stop=(j == 3))
                # cols [o|i|f|g]: s~ = tanh(pre/2) = 2*sig(pre)-1; T = tanh(g)
                # state D = 2c: M = (s~[i,f]+1)*[T|D]; D' = M2/2 + M1;
                # tch = tanh(D/2) = tanh(c); h~ = (s~o+1)*tch = 2h
                nc.scalar.activation(sigD[:, 0:32], psr[:], AF.Tanh,
                                     scale=0.5)
                nc.vector.scalar_tensor_tensor(
                    out=Mt[:], in0=sigD[:, 8:24], scalar=1.0,
                    in1=sigD[:, 24:40], op0=OP.add, op1=OP.mult)
                nc.vector.scalar_tensor_tensor(
                    out=sigD[:, 32:40], in0=Mt[:, 8:16], scalar=0.5,
                    in1=Mt[:, 0:8], op0=OP.mult, op1=OP.add)
                tch = rp.tile([128, 8], F32, tag="tch")
                nc.scalar.activation(tch[:], sigD[:, 32:40], AF.Tanh,
                                     scale=0.5)
                nc.vector.scalar_tensor_tensor(
                    out=hseq[:, 8 * t:8 * t + 8], in0=sigD[:, 0:8],
                    scalar=1.0, in1=tch[:], op0=OP.add, op1=OP.mult)

                for item in side.get(t, ()):
                    if item[0] == "tile":
                        finish_tile(item[1])
                    elif item[0] == "gemm0b":
                        gemm_range(64, 64, item[1])
                    elif item[0] in ("gemm_a", "gemm_b"):
                        t0, ntl = _CHUNKS[item[1]]
                        fn = gemm_a if item[0] == "gemm_a" else gemm_b
                        fn(128 * t0, min(128 * ntl, NT - 128 * t0), item[2])
                    elif item[0] == "probdma":
                        nc.sync.dma_start(
                            prob[:].rearrange("(t p) -> p t", p=128)[:, 0:12],
                            prob_sb[:, 0:12])
                    else:
                        out_tile(item[1])

            for t in late_out:
                out_tile(t)

            nc.sync.dma_start(
                prob[:].rearrange("(t p) -> p t", p=128)[:, 12:13],
                prob_sb[:, 12:13])

    nc.compile()
    return nc


_CACHED = None


def _get_kernel():
    global _CACHED
    if _CACHED is None:
        _CACHED = build_kernel()
    return _CACHED


def _prep_shared(pro_embed, W_ih, W_hh, b_ih, b_hh, W_out, b_out):
    wx_h = np.empty((128, 1024), np.float32)
    whh_h = np.empty((128, 512), np.float32)
    bias_h = np.empty((128, 4), np.float32)
    for j, g in enumerate(_GATE_SRC):
        blk = slice(g * 128, (g + 1) * 128)
        scx = 2.0 if j == 3 else 1.0   # g-gate preact x2: tanh((2g)/2)=tanh(g)
        sch = 1.0 if j == 3 else 0.5   # /2 for h~ = 2h feedback
        wx_h[:, j * 128:(j + 1) * 128] = scx * W_ih[blk, 0:128].T
        wx_h[:, 512 + j * 128:512 + (j + 1) * 128] = scx * W_ih[blk, 128:256].T
        whh_h[:, j * 128:(j + 1) * 128] = sch * W_hh[blk, :].T
        bias_h[:, j] = scx * (b_ih[blk] + b_hh[blk])
    wb_h = np.empty((PRO_NUM, WB), np.float32)
    wb_h[:, :H] = 0.5 * W_out
    wb_h[:, H] = 0.5 * b_out
    return dict(
        emb=np.ascontiguousarray(pro_embed, np.float32),
        wx=np.ascontiguousarray(wx_h),
        whh=np.ascontiguousarray(whh_h),
        bsum=np.ascontiguousarray(bias_h),
        wb=wb_h,
    )


def kernel(X, y, pro_embed, W_ih, W_hh, b_ih, b_hh, W_out, b_out, _trace=False,
           **_):
    X = np.asarray(X, np.int32)
    y = np.asarray(y, np.int32)
    shared = _prep_shared(np.asarray(pro_embed, np.float32),
                          np.asarray(W_ih, np.float32),
                          np.asarray(W_hh, np.float32),
                          np.asarray(b_ih, np.float32),
                          np.asarray(b_hh, np.float32),
                          np.asarray(W_out, np.float32),
                          np.asarray(b_out, np.float32))
    XT = X.T  # [200, 64]
    YT = y.T
    in_maps = []
    for c in range(N_CORES):
        cols = slice(c * BS, (c + 1) * BS)
        xtp = np.zeros((209, BS), np.int32)
        xtp[:S] = XT[:, cols]
        ytp = np.zeros((208, BS), np.int32)
        ytp[:S] = YT[:, cols]
        in_maps.append(dict(xt=xtp, yt=ytp, **shared))

    nc = _get_kernel()
    res = run_bass_kernel_spmd(nc, in_maps, core_ids=list(range(N_CORES)),
                               trace=_trace)
    out = np.empty((B, S - 1), np.float32)
    for c in range(N_CORES):
        flat = res.results[c]["prob"][:NOUT].reshape(S - 1, BS)
        out[c * BS:(c + 1) * BS, :] = flat.T
    if _trace:
        return out, res
    return out



# revision 22
# speedup vs baseline: 1.0899x; 1.0166x over previous
"""DKT-PEBG kernel for Trainium2 (8 NeuronCores, batch-parallel).

Model: embedding lookup -> masked concat -> LSTM(128) -> per-token output
probability via gathered W_out rows (avoids materializing [B,S,10000]).

Sharding: data-parallel over batch. Core c handles batch rows [8c, 8c+8).
No collectives; host splits inputs / concatenates outputs.

Shapes (hardcoded): B=64, S=200, E=H=128, PRO_NUM=10000.

Recurrence trick: gate order [i,f,o,g] with the g-gate pre-activation
prescaled by 2 on the host, so one Sigmoid over all 4 gates suffices:
tanh(g) = 2*sigmoid(2g) - 1. Input-GEMM chunks and gathers are emitted
interleaved with the first recurrence steps so the scheduler pipelines
them instead of serializing ~45us of startup.
"""

import numpy as np

import concourse.bass as bass
import concourse.bacc as bacc
import concourse.mybir as mybir
import concourse.tile as tile
from concourse.bass_utils import run_bass_kernel_spmd
from concourse.masks import make_identity

B, S = 64, 200
E = 128
H = 128
PRO_NUM = 10000
N_CORES = 8
BS = B // N_CORES              # 8 batch rows per core
NT = BS * S                    # 1600 tokens per core, token n = 8*s + b
NTILES = 13                    # ceil(1600/128); tile 12 has 64 valid tokens
NOUT = BS * (S - 1)            # 1592 output tokens
WB = H + 1                     # gathered W_out row + bias
F32 = mybir.dt.float32
BF16 = mybir.dt.bfloat16
I32 = mybir.dt.int32

_GATE_SRC = (3, 0, 1, 2)       # col blocks [o, i, f, g] <- W_ih row blocks (i,f,g,o)

# input-GEMM chunks in tiles: (first_tile, n_tiles); chunk 0 small so the
# recurrence can start early
_CHUNKS = ((0, 1), (1, 2), (3, 2), (5, 4), (9, 4))


def _tok_w(t):
    return 128 if t < NTILES - 1 else NT - 128 * (NTILES - 1)


def _out_w(t):
    return 128 if t < NTILES - 1 else NOUT - 128 * (NTILES - 1)


def build_kernel():
    nc = bacc.Bacc("TRN2", target_bir_lowering=False, debug=False,
                   num_devices=N_CORES)

    # ---- I/O ----
    xt = nc.dram_tensor("xt", [209, BS], I32, kind="ExternalInput")   # X.T slice, padded
    yt = nc.dram_tensor("yt", [208, BS], I32, kind="ExternalInput")   # y.T slice, padded
    emb = nc.dram_tensor("emb", [PRO_NUM, E], F32, kind="ExternalInput")
    wx = nc.dram_tensor("wx", [128, 1024], F32, kind="ExternalInput")  # W_ih.T blocks [A|B]
    whh = nc.dram_tensor("whh", [128, 512], BF16, kind="ExternalInput")  # W_hh.T blocks
    bsum = nc.dram_tensor("bsum", [128, 4], F32, kind="ExternalInput")  # b_ih+b_hh blocks
    wb = nc.dram_tensor("wb", [PRO_NUM, WB], F32, kind="ExternalInput")  # [W_out | b_out]
    prob = nc.dram_tensor("prob", [NTILES * 128], F32, kind="ExternalOutput")

    AF = mybir.ActivationFunctionType
    OP = mybir.AluOpType

    with tile.TileContext(nc) as tc:
        with (
            tc.tile_pool(name="persist", bufs=1) as pp,
            tc.tile_pool(name="work", bufs=4) as wp,
            tc.tile_pool(name="exp", bufs=13) as expool,
            tc.tile_pool(name="rec", bufs=4) as rp,
            tc.tile_pool(name="ps_tr", bufs=2, space="PSUM") as ps_tr,
            tc.tile_pool(name="ps_mm", bufs=3, space="PSUM") as ps_mm,
            tc.tile_pool(name="ps_rec", bufs=2, space="PSUM") as ps_rec,
            tc.tile_pool(name="ps_h", bufs=1, space="PSUM") as ps_h,
        ):
            # ---- persistent SBUF ----
            ident = pp.tile([128, 128], F32, tag="ident")
            identb = pp.tile([128, 128], BF16, tag="identb")
            wx_sb = pp.tile([128, 1024], F32, tag="wx_sb")
            whh_sb = pp.tile([128, 512], BF16, tag="whh_sb")
            bias_sb = pp.tile([128, 4], F32, tag="bias_sb")
            ix_all = pp.tile([128, NTILES], I32, tag="ix_all")
            ixs_all = pp.tile([128, NTILES], I32, tag="ixs_all")
            y_all = pp.tile([128, NTILES], I32, tag="y_all")
            y_f = pp.tile([128, NTILES], F32, tag="y_f")
            m1 = pp.tile([128, NTILES], F32, tag="m1")
            m2 = pp.tile([128, NTILES], F32, tag="m2")
            ixm1 = pp.tile([128, NTILES], I32, tag="ixm1")
            ixs_f = pp.tile([128, NTILES], F32, tag="ixs_f")
            mnz = pp.tile([128, NTILES], F32, tag="mnz")
            xaT = pp.tile([128, NT], F32, tag="xaT")
            xbT = pp.tile([128, NT], F32, tag="xbT")
            xgb = pp.tile([128, 32 * S], BF16, tag="xgb")
            hseq = pp.tile([128, NT], BF16, tag="hseq")
            wgb_all = pp.tile([128, NTILES * WB], F32, tag="wgb_all")
            sigD = pp.tile([128, 40], F32, tag="sigD")
            Mt = pp.tile([128, 16], F32, tag="Mt")
            prob_sb = pp.tile([128, NTILES], F32, tag="prob_sb")

            make_identity(nc, ident[:])
            make_identity(nc, identb[:])
            nc.gpsimd.memset(prob_sb[:], 0.0)
            nc.gpsimd.memset(sigD[:, 32:40], 0.0)

            # ---- loads ----
            xt_flat = xt[:].rearrange("s b -> (s b)")
            yt_flat = yt[:].rearrange("s b -> (s b)")
            nc.sync.dma_start(
                ix_all[:], xt_flat[0:1664].rearrange("(t p) -> p t", p=128))
            nc.sync.dma_start(
                y_all[:], yt_flat[0:1664].rearrange("(t p) -> p t", p=128))
            nc.sync.dma_start(
                ixs_all[:], xt_flat[8:1672].rearrange("(t p) -> p t", p=128))
            nc.sync.dma_start(wx_sb[:], wx[:])
            nc.sync.dma_start(whh_sb[:], whh[:])
            nc.sync.dma_start(bias_sb[:], bsum[:])

            # warm the ACT sigmoid/tanh table set off the critical path
            warm = wp.tile([1, 1], F32, tag="warm")
            nc.scalar.activation(warm[:], ident[0:1, 0:1], AF.Tanh)

            # warm the PE HAM clock gate so the startup GEMM runs at full rate
            for _ in range(10):
                pwm = ps_mm.tile([128, 512], F32, tag="psg")
                nc.tensor.matmul(pwm[:, 0:128], ident[:], ident[:],
                                 start=True, stop=True)

            # masks: m1 = (y==0), m2 = (y==1); padding y==-1 -> 0,0
            nc.vector.tensor_copy(y_f[:], y_all[:])
            nc.vector.tensor_scalar(m1[:], y_f[:], 0.0, None, op0=OP.is_equal)
            nc.vector.tensor_scalar(m2[:], y_f[:], 1.0, None, op0=OP.is_equal)

            ex_tiles = {}

            def gather_tile(t):
                w = _tok_w(t)
                ex_t = expool.tile([128, E], F32, tag="ex")
                ex_tiles[t] = ex_t
                nc.gpsimd.indirect_dma_start(
                    out=ex_t[0:w, :], out_offset=None, in_=emb[:],
                    in_offset=bass.IndirectOffsetOnAxis(
                        ap=ix_all[0:w, t:t + 1], axis=0))

            def finish_tile(t):
                """mask + transpose gathered tile t into xaT/xbT"""
                w = _tok_w(t)
                ex_t = ex_tiles.pop(t)
                xa_t = wp.tile([128, E], F32, tag="xa")
                xb_t = wp.tile([128, E], F32, tag="xb")
                nc.vector.tensor_scalar(xa_t[0:w, :], ex_t[0:w, :],
                                        m1[0:w, t:t + 1], None, op0=OP.mult)
                nc.vector.tensor_scalar(xb_t[0:w, :], ex_t[0:w, :],
                                        m2[0:w, t:t + 1], None, op0=OP.mult)
                psa = ps_tr.tile([128, 128], F32, tag="psa")
                nc.tensor.transpose(psa[:, 0:w], xa_t[0:w, :], ident[0:w, 0:w])
                nc.vector.tensor_copy(xaT[:, 128 * t:128 * t + w], psa[:, 0:w])
                psb = ps_tr.tile([128, 128], F32, tag="psa")
                nc.tensor.transpose(psb[:, 0:w], xb_t[0:w, :], ident[0:w, 0:w])
                nc.vector.tensor_copy(xbT[:, 128 * t:128 * t + w], psb[:, 0:w])

            pending_psg = {}

            def process_tile(t):
                gather_tile(t)
                finish_tile(t)

            def gemm_a(base, w, j):
                psg = ps_mm.tile([128, 512], F32, tag="psg")
                pending_psg[(base, j)] = psg
                nc.tensor.matmul(
                    psg[:, 0:w], wx_sb[:, 128 * j:128 * (j + 1)],
                    xaT[:, base:base + w], start=True, stop=False)

            def gemm_b(base, w, j):
                psg = pending_psg.pop((base, j))
                nc.tensor.matmul(
                    psg[:, 0:w], wx_sb[:, 512 + 128 * j:512 + 128 * (j + 1)],
                    xbT[:, base:base + w], start=False, stop=True)
                dst = xgb[:, 4 * base: 4 * base + 32 * (w // 8)] \
                    .rearrange("p (q x) -> p q x", x=32)[:, :, 8 * j:8 * j + 8]
                src = psg[:, 0:w].rearrange("p (q x) -> p q x", x=8)
                nc.vector.tensor_scalar(dst, src, bias_sb[:, j:j + 1], None,
                                        op0=OP.add)

            def gemm_range(base, w, j):
                gemm_a(base, w, j)
                gemm_b(base, w, j)

            def gather_wb(t):
                w = _out_w(t)
                nc.gpsimd.indirect_dma_start(
                    out=wgb_all[0:w, WB * t:WB * (t + 1)], out_offset=None,
                    in_=wb[:],
                    in_offset=bass.IndirectOffsetOnAxis(
                        ap=ixm1[0:w, t:t + 1], axis=0))

            def out_tile(t):
                '''prob = sigmoid(h . W_out[idx] + b_out[idx]) * (X != 0)'''
                w = _out_w(t)
                pst = ps_h.tile([128, 128], BF16, tag="psh")
                nc.tensor.transpose(pst[0:w, :], hseq[:, 128 * t:128 * t + w],
                                    identb[:])
                hw_t = wp.tile([128, 128], F32, tag="hw")
                d_t = wp.tile([128, 1], F32, tag="d")
                nc.vector.scalar_tensor_tensor(
                    out=hw_t[0:w, :], in0=pst[0:w, :], scalar=1.0,
                    in1=wgb_all[0:w, WB * t:WB * t + H],
                    op0=OP.mult, op1=OP.mult, accum_out=d_t[0:w, :])
                p_t = wp.tile([128, 1], F32, tag="p")
                nc.scalar.activation(p_t[0:w, :], d_t[0:w, :], AF.Tanh,
                                     bias=wgb_all[0:w, WB * t + H:WB * (t + 1)],
                                     scale=0.5)
                nc.vector.scalar_tensor_tensor(
                    out=prob_sb[0:w, t:t + 1], in0=p_t[0:w, :], scalar=1.0,
                    in1=mnz[0:w, t:t + 1], op0=OP.add, op1=OP.mult)

            # interleave schedule: step index -> list of thunks.
            # chunk c tokens start at step 16*_CHUNKS[c][0]; stay ahead of it.
            side = {}
            tile_steps = {1: (1, 2), 2: (11, 13), 3: (30, 34, 38, 42),
                          4: (60, 64, 68, 72)}
            gemm_steps = {1: 3, 2: 16, 3: 45, 4: 76}
            for j in range(4):                  # second half of tile 0
                side.setdefault(2 + j, []).append(("gemm0b", j))
            for c in (1, 2, 3, 4):
                t0, ntl = _CHUNKS[c]
                for k in range(ntl):
                    side.setdefault(tile_steps[c][k], []).append(
                        ("tile", t0 + k))
                for j in range(4):
                    s0 = gemm_steps[c] + 2 * j
                    side.setdefault(s0, []).append(("gemm_a", c, j))
                    side.setdefault(s0 + 1, []).append(("gemm_b", c, j))
            late_out = []
            for t in range(NTILES):             # output tiles once h is ready
                # tile t reads h(s) up to s = 16t + (w-1)//8
                smax = 16 * t + (_out_w(t) - 1) // 8
                step = max(smax + 1, 140 + 4 * t)
                if step <= S - 1:
                    side.setdefault(step, []).append(("out", t))
                else:
                    late_out.append(t)
            side.setdefault(196, []).append(("probdma",))

            # ---- chunk 0 (first 64 tokens) then the recurrence ----
            process_tile(0)
            for j in range(4):
                gemm_range(0, 64, j)

            # index prep for the W_out gathers (off the sigma(0) path)
            nc.vector.tensor_scalar(ixm1[:], ixs_all[:], 1, 0,
                                    op0=OP.subtract, op1=OP.max)
            nc.vector.tensor_copy(ixs_f[:], ixs_all[:])
            nc.vector.tensor_scalar(mnz[:], ixs_f[:], 0.0, 0.5,
                                    op0=OP.not_equal, op1=OP.mult)

            # queue every remaining gather now; the Pool engine drains them
            # in the background while the recurrence runs on PE/ACT/DVE
            for t in range(1, NTILES):
                gather_tile(t)
            for t in range(NTILES):
                gather_wb(t)

            for t in range(S):
                psr = ps_rec.tile([128, 32], F32, tag="psr")
                nc.tensor.matmul(psr[:], identb[:], xgb[:, 32 * t:32 * t + 32],
                                 start=True, stop=(t == 0))
                if t > 0:
                    hprev = hseq[:, 8 * (t - 1):8 * t]
                    for j in range(4):
                        nc.tensor.matmul(
                            psr[:, 8 * j:8 * j + 8],
                            whh_sb[:, 128 * j:128 * (j + 1)], hprev,
                            start=False, stop=(j == 3))
                # cols [o|i|f|g]: s~ = tanh(pre/2) = 2*sig(pre)-1; T = tanh(g)
                # state D = 2c: M = (s~[i,f]+1)*[T|D]; D' = M2/2 + M1;
                # tch = tanh(D/2) = tanh(c); h~ = (s~o+1)*tch = 2h
                nc.scalar.activation(sigD[:, 0:32], psr[:], AF.Tanh,
                                     scale=0.5)
                nc.vector.scalar_tensor_tensor(
                    out=Mt[:], in0=sigD[:, 8:24], scalar=1.0,
                    in1=sigD[:, 24:40], op0=OP.add, op1=OP.mult)
                nc.vector.scalar_tensor_tensor(
                    out=sigD[:, 32:40], in0=Mt[:, 8:16], scalar=0.5,
                    in1=Mt[:, 0:8], op0=OP.mult, op1=OP.add)
                tch = rp.tile([128, 8], F32, tag="tch")
                nc.scalar.activation(tch[:], sigD[:, 32:40], AF.Tanh,
                                     scale=0.5)
                nc.vector.scalar_tensor_tensor(
                    out=hseq[:, 8 * t:8 * t + 8], in0=sigD[:, 0:8],
                    scalar=1.0, in1=tch[:], op0=OP.add, op1=OP.mult)

                for item in side.get(t, ()):
                    if item[0] == "tile":
                        finish_tile(item[1])
                    elif item[0] == "gemm0b":
                        gemm_range(64, 64, item[1])
                    elif item[0] in ("gemm_a", "gemm_b"):
                        t0, ntl = _CHUNKS[item[1]]
                        fn = gemm_a if item[0] == "gemm_a" else gemm_b
                        fn(128 * t0, min(128 * ntl, NT - 128 * t0), item[2])
                    elif item[0] == "probdma":
                        nc.sync.dma_start(
                            prob[:].rearrange("(t p) -> p t", p=128)[:, 0:12],
                            prob_sb[:, 0:12])
                    else:
                        out_tile(item[1])

            for t in late_out:
                out_tile(t)

            nc.sync.dma_start(
                prob[:].rearrange("(t p) -> p t", p=128)[:, 12:13],
                prob_sb[:, 12:13])

    nc.compile()
    return nc


_CACHED = None


def _get_kernel():
    global _CACHED
    if _CACHED is None:
        _CACHED = build_kernel()
    return _CACHED


def _prep_shared(pro_embed, W_ih, W_hh, b_ih, b_hh, W_out, b_out):
    wx_h = np.empty((128, 1024), np.float32)
    whh_h = np.empty((128, 512), np.float32)
    bias_h = np.empty((128, 4), np.float32)
    for j, g in enumerate(_GATE_SRC):
        blk = slice(g * 128, (g + 1) * 128)
        scx = 2.0 if j == 3 else 1.0   # g-gate preact x2: tanh((2g)/2)=tanh(g)
        sch = 1.0 if j == 3 else 0.5   # /2 for h~ = 2h feedback
        wx_h[:, j * 128:(j + 1) * 128] = scx * W_ih[blk, 0:128].T
        wx_h[:, 512 + j * 128:512 + (j + 1) * 128] = scx * W_ih[blk, 128:256].T
        whh_h[:, j * 128:(j + 1) * 128] = sch * W_hh[blk, :].T
        bias_h[:, j] = scx * (b_ih[blk] + b_hh[blk])
    wb_h = np.empty((PRO_NUM, WB), np.float32)
    wb_h[:, :H] = 0.5 * W_out
    wb_h[:, H] = 0.5 * b_out
    import ml_dtypes
    return dict(
        emb=np.ascontiguousarray(pro_embed, np.float32),
        wx=np.ascontiguousarray(wx_h),
        whh=np.ascontiguousarray(whh_h).astype(ml_dtypes.bfloat16),
        bsum=np.ascontiguousarray(bias_h),
        wb=wb_h,
    )


def kernel(X, y, pro_embed, W_ih, W_hh, b_ih, b_hh, W_out, b_out, _trace=False,
           **_):
    X = np.asarray(X, np.int32)
    y = np.asarray(y, np.int32)
    shared = _prep_shared(np.asarray(pro_embed, np.float32),
                          np.asarray(W_ih, np.float32),
                          np.asarray(W_hh, np.float32),
                          np.asarray(b_ih, np.float32),
                          np.asarray(b_hh, np.float32),
                          np.asarray(W_out, np.float32),
                          np.asarray(b_out, np.float32))
    XT = X.T  # [200, 64]
    YT = y.T
    in_maps = []
    for c in range(N_CORES):
        cols = slice(c * BS, (c + 1) * BS)
        xtp = np.zeros((209, BS), np.int32)
        xtp[:S] = XT[:, cols]
        ytp = np.zeros((208, BS), np.int32)
        ytp[:S] = YT[:, cols]
        in_maps.append(dict(xt=xtp, yt=ytp, **shared))

    nc = _get_kernel()
    res = run_bass_kernel_spmd(nc, in_maps, core_ids=list(range(N_CORES)),
                               trace=_trace)
    out = np.empty((B, S - 1), np.float32)
    for c in range(N_CORES):
        flat = res.results[c]["prob"][:NOUT].reshape(S - 1, BS)
        out[c * BS:(c + 1) * BS, :] = flat.T
    if _trace:
        return out, res
    return out



# revision 23
# speedup vs baseline: 1.0990x; 1.0083x over previous
"""DKT-PEBG kernel for Trainium2 (8 NeuronCores, batch-parallel).

Model: embedding lookup -> masked concat -> LSTM(128) -> per-token output
probability via gathered W_out rows (avoids materializing [B,S,10000]).

Sharding: data-parallel over batch. Core c handles batch rows [8c, 8c+8).
No collectives; host splits inputs / concatenates outputs.

Shapes (hardcoded): B=64, S=200, E=H=128, PRO_NUM=10000.

Recurrence trick: gate order [i,f,o,g] with the g-gate pre-activation
prescaled by 2 on the host, so one Sigmoid over all 4 gates suffices:
tanh(g) = 2*sigmoid(2g) - 1. Input-GEMM chunks and gathers are emitted
interleaved with the first recurrence steps so the scheduler pipelines
them instead of serializing ~45us of startup.
"""

import numpy as np

import concourse.bass as bass
import concourse.bacc as bacc
import concourse.mybir as mybir
import concourse.tile as tile
from concourse.bass_utils import run_bass_kernel_spmd
from concourse.masks import make_identity

B, S = 64, 200
E = 128
H = 128
PRO_NUM = 10000
N_CORES = 8
BS = B // N_CORES              # 8 batch rows per core
NT = BS * S                    # 1600 tokens per core, token n = 8*s + b
NTILES = 13                    # ceil(1600/128); tile 12 has 64 valid tokens
NOUT = BS * (S - 1)            # 1592 output tokens
WB = H + 1                     # gathered W_out row + bias
F32 = mybir.dt.float32
BF16 = mybir.dt.bfloat16
I32 = mybir.dt.int32

_GATE_SRC = (3, 0, 1, 2)       # col blocks [o, i, f, g] <- W_ih row blocks (i,f,g,o)

# input-GEMM chunks in tiles: (first_tile, n_tiles); chunk 0 small so the
# recurrence can start early
_CHUNKS = ((0, 1), (1, 2), (3, 2), (5, 4), (9, 4))


def _tok_w(t):
    return 128 if t < NTILES - 1 else NT - 128 * (NTILES - 1)


def _out_w(t):
    return 128 if t < NTILES - 1 else NOUT - 128 * (NTILES - 1)


def build_kernel():
    nc = bacc.Bacc("TRN2", target_bir_lowering=False, debug=False,
                   num_devices=N_CORES)

    # ---- I/O ----
    xt = nc.dram_tensor("xt", [209, BS], I32, kind="ExternalInput")   # X.T slice, padded
    yt = nc.dram_tensor("yt", [208, BS], I32, kind="ExternalInput")   # y.T slice, padded
    emb = nc.dram_tensor("emb", [PRO_NUM, E], F32, kind="ExternalInput")
    wx = nc.dram_tensor("wx", [128, 1024], BF16, kind="ExternalInput")  # W_ih.T blocks [A|B]
    whh = nc.dram_tensor("whh", [128, 512], BF16, kind="ExternalInput")  # W_hh.T blocks
    bsum = nc.dram_tensor("bsum", [128, 4], F32, kind="ExternalInput")  # b_ih+b_hh blocks
    wb = nc.dram_tensor("wb", [PRO_NUM, WB], F32, kind="ExternalInput")  # [W_out | b_out]
    prob = nc.dram_tensor("prob", [NTILES * 128], F32, kind="ExternalOutput")

    AF = mybir.ActivationFunctionType
    OP = mybir.AluOpType

    with tile.TileContext(nc) as tc:
        with (
            tc.tile_pool(name="persist", bufs=1) as pp,
            tc.tile_pool(name="work", bufs=4) as wp,
            tc.tile_pool(name="exp", bufs=13) as expool,
            tc.tile_pool(name="rec", bufs=4) as rp,
            tc.tile_pool(name="ps_tr", bufs=2, space="PSUM") as ps_tr,
            tc.tile_pool(name="ps_mm", bufs=3, space="PSUM") as ps_mm,
            tc.tile_pool(name="ps_rec", bufs=2, space="PSUM") as ps_rec,
            tc.tile_pool(name="ps_h", bufs=1, space="PSUM") as ps_h,
        ):
            # ---- persistent SBUF ----
            ident = pp.tile([128, 128], F32, tag="ident")
            identb = pp.tile([128, 128], BF16, tag="identb")
            wx_sb = pp.tile([128, 1024], BF16, tag="wx_sb")
            whh_sb = pp.tile([128, 512], BF16, tag="whh_sb")
            bias_sb = pp.tile([128, 4], F32, tag="bias_sb")
            ix_all = pp.tile([128, NTILES], I32, tag="ix_all")
            ixs_all = pp.tile([128, NTILES], I32, tag="ixs_all")
            y_all = pp.tile([128, NTILES], I32, tag="y_all")
            y_f = pp.tile([128, NTILES], F32, tag="y_f")
            m1 = pp.tile([128, NTILES], F32, tag="m1")
            m2 = pp.tile([128, NTILES], F32, tag="m2")
            ixm1 = pp.tile([128, NTILES], I32, tag="ixm1")
            ixs_f = pp.tile([128, NTILES], F32, tag="ixs_f")
            mnz = pp.tile([128, NTILES], F32, tag="mnz")
            xaT = pp.tile([128, NT], BF16, tag="xaT")
            xbT = pp.tile([128, NT], BF16, tag="xbT")
            xgb = pp.tile([128, 32 * S], BF16, tag="xgb")
            hseq = pp.tile([128, NT], BF16, tag="hseq")
            wgb_all = pp.tile([128, NTILES * WB], F32, tag="wgb_all")
            sigD = pp.tile([128, 40], F32, tag="sigD")
            Mt = pp.tile([128, 16], F32, tag="Mt")
            prob_sb = pp.tile([128, NTILES], F32, tag="prob_sb")

            make_identity(nc, ident[:])
            make_identity(nc, identb[:])
            nc.gpsimd.memset(prob_sb[:], 0.0)
            nc.gpsimd.memset(sigD[:, 32:40], 0.0)

            # ---- loads ----
            xt_flat = xt[:].rearrange("s b -> (s b)")
            yt_flat = yt[:].rearrange("s b -> (s b)")
            nc.sync.dma_start(
                ix_all[:], xt_flat[0:1664].rearrange("(t p) -> p t", p=128))
            nc.sync.dma_start(
                y_all[:], yt_flat[0:1664].rearrange("(t p) -> p t", p=128))
            nc.sync.dma_start(
                ixs_all[:], xt_flat[8:1672].rearrange("(t p) -> p t", p=128))
            nc.sync.dma_start(wx_sb[:], wx[:])
            nc.sync.dma_start(whh_sb[:], whh[:])
            nc.sync.dma_start(bias_sb[:], bsum[:])

            # warm the ACT sigmoid/tanh table set off the critical path
            warm = wp.tile([1, 1], F32, tag="warm")
            nc.scalar.activation(warm[:], ident[0:1, 0:1], AF.Tanh)

            # warm the PE HAM clock gate so the startup GEMM runs at full rate
            for _ in range(10):
                pwm = ps_mm.tile([128, 512], F32, tag="psg")
                nc.tensor.matmul(pwm[:, 0:128], ident[:], ident[:],
                                 start=True, stop=True)

            # masks: m1 = (y==0), m2 = (y==1); padding y==-1 -> 0,0
            nc.vector.tensor_copy(y_f[:], y_all[:])
            nc.vector.tensor_scalar(m1[:], y_f[:], 0.0, None, op0=OP.is_equal)
            nc.vector.tensor_scalar(m2[:], y_f[:], 1.0, None, op0=OP.is_equal)

            ex_tiles = {}

            def gather_tile(t):
                w = _tok_w(t)
                ex_t = expool.tile([128, E], F32, tag="ex")
                ex_tiles[t] = ex_t
                nc.gpsimd.indirect_dma_start(
                    out=ex_t[0:w, :], out_offset=None, in_=emb[:],
                    in_offset=bass.IndirectOffsetOnAxis(
                        ap=ix_all[0:w, t:t + 1], axis=0))

            def finish_tile(t):
                """mask + transpose gathered tile t into xaT/xbT"""
                w = _tok_w(t)
                ex_t = ex_tiles.pop(t)
                xa_t = wp.tile([128, E], F32, tag="xa")
                xb_t = wp.tile([128, E], F32, tag="xb")
                nc.vector.tensor_scalar(xa_t[0:w, :], ex_t[0:w, :],
                                        m1[0:w, t:t + 1], None, op0=OP.mult)
                nc.vector.tensor_scalar(xb_t[0:w, :], ex_t[0:w, :],
                                        m2[0:w, t:t + 1], None, op0=OP.mult)
                psa = ps_tr.tile([128, 128], F32, tag="psa")
                nc.tensor.transpose(psa[:, 0:w], xa_t[0:w, :], ident[0:w, 0:w])
                nc.vector.tensor_copy(xaT[:, 128 * t:128 * t + w], psa[:, 0:w])
                psb = ps_tr.tile([128, 128], F32, tag="psa")
                nc.tensor.transpose(psb[:, 0:w], xb_t[0:w, :], ident[0:w, 0:w])
                nc.vector.tensor_copy(xbT[:, 128 * t:128 * t + w], psb[:, 0:w])

            pending_psg = {}

            def process_tile(t):
                gather_tile(t)
                finish_tile(t)

            def gemm_a(base, w, j):
                psg = ps_mm.tile([128, 512], F32, tag="psg")
                pending_psg[(base, j)] = psg
                nc.tensor.matmul(
                    psg[:, 0:w], wx_sb[:, 128 * j:128 * (j + 1)],
                    xaT[:, base:base + w], start=True, stop=False)

            def gemm_b(base, w, j):
                psg = pending_psg.pop((base, j))
                nc.tensor.matmul(
                    psg[:, 0:w], wx_sb[:, 512 + 128 * j:512 + 128 * (j + 1)],
                    xbT[:, base:base + w], start=False, stop=True)
                dst = xgb[:, 4 * base: 4 * base + 32 * (w // 8)] \
                    .rearrange("p (q x) -> p q x", x=32)[:, :, 8 * j:8 * j + 8]
                src = psg[:, 0:w].rearrange("p (q x) -> p q x", x=8)
                nc.vector.tensor_scalar(dst, src, bias_sb[:, j:j + 1], None,
                                        op0=OP.add)

            def gemm_range(base, w, j):
                gemm_a(base, w, j)
                gemm_b(base, w, j)

            def gather_wb(t):
                w = _out_w(t)
                nc.gpsimd.indirect_dma_start(
                    out=wgb_all[0:w, WB * t:WB * (t + 1)], out_offset=None,
                    in_=wb[:],
                    in_offset=bass.IndirectOffsetOnAxis(
                        ap=ixm1[0:w, t:t + 1], axis=0))

            def out_tile(t):
                '''prob = sigmoid(h . W_out[idx] + b_out[idx]) * (X != 0)'''
                w = _out_w(t)
                pst = ps_h.tile([128, 128], BF16, tag="psh")
                nc.tensor.transpose(pst[0:w, :], hseq[:, 128 * t:128 * t + w],
                                    identb[:])
                hw_t = wp.tile([128, 128], F32, tag="hw")
                d_t = wp.tile([128, 1], F32, tag="d")
                nc.vector.scalar_tensor_tensor(
                    out=hw_t[0:w, :], in0=pst[0:w, :], scalar=1.0,
                    in1=wgb_all[0:w, WB * t:WB * t + H],
                    op0=OP.mult, op1=OP.mult, accum_out=d_t[0:w, :])
                p_t = wp.tile([128, 1], F32, tag="p")
                nc.scalar.activation(p_t[0:w, :], d_t[0:w, :], AF.Tanh,
                                     bias=wgb_all[0:w, WB * t + H:WB * (t + 1)],
                                     scale=0.5)
                nc.vector.scalar_tensor_tensor(
                    out=prob_sb[0:w, t:t + 1], in0=p_t[0:w, :], scalar=1.0,
                    in1=mnz[0:w, t:t + 1], op0=OP.add, op1=OP.mult)

            # interleave schedule: step index -> list of thunks.
            # chunk c tokens start at step 16*_CHUNKS[c][0]; stay ahead of it.
            side = {}
            tile_steps = {1: (1, 2), 2: (11, 13), 3: (30, 34, 38, 42),
                          4: (60, 64, 68, 72)}
            gemm_steps = {1: 3, 2: 16, 3: 45, 4: 76}
            gemm_gap = {1: 3, 2: 2, 3: 2, 4: 2}
            for j in range(4):                  # second half of tile 0
                side.setdefault(2 + j, []).append(("gemm0b", j))
            for c in (1, 2, 3, 4):
                t0, ntl = _CHUNKS[c]
                for k in range(ntl):
                    side.setdefault(tile_steps[c][k], []).append(
                        ("tile", t0 + k))
                for j in range(4):
                    s0 = gemm_steps[c] + gemm_gap[c] * j
                    side.setdefault(s0, []).append(("gemm_a", c, j))
                    side.setdefault(s0 + 1, []).append(("gemm_b", c, j))
            late_out = []
            for t in range(NTILES):             # output tiles once h is ready
                # tile t reads h(s) up to s = 16t + (w-1)//8
                smax = 16 * t + (_out_w(t) - 1) // 8
                step = max(smax + 1, 140 + 4 * t)
                if step <= S - 1:
                    side.setdefault(step, []).append(("out", t))
                else:
                    late_out.append(t)
            side.setdefault(196, []).append(("probdma",))

            # ---- chunk 0 (first 64 tokens) then the recurrence ----
            process_tile(0)
            for j in range(4):
                gemm_range(0, 64, j)

            # index prep for the W_out gathers (off the sigma(0) path)
            nc.vector.tensor_scalar(ixm1[:], ixs_all[:], 1, 0,
                                    op0=OP.subtract, op1=OP.max)
            nc.vector.tensor_copy(ixs_f[:], ixs_all[:])
            nc.vector.tensor_scalar(mnz[:], ixs_f[:], 0.0, 0.5,
                                    op0=OP.not_equal, op1=OP.mult)

            # queue every remaining gather now; the Pool engine drains them
            # in the background while the recurrence runs on PE/ACT/DVE
            for t in range(1, NTILES):
                gather_tile(t)
            for t in range(NTILES):
                gather_wb(t)

            for t in range(S):
                psr = ps_rec.tile([128, 32], F32, tag="psr")
                nc.tensor.matmul(psr[:], identb[:], xgb[:, 32 * t:32 * t + 32],
                                 start=True, stop=(t == 0))
                if t > 0:
                    hprev = hseq[:, 8 * (t - 1):8 * t]
                    for j in range(4):
                        nc.tensor.matmul(
                            psr[:, 8 * j:8 * j + 8],
                            whh_sb[:, 128 * j:128 * (j + 1)], hprev,
                            start=False, stop=(j == 3))
                # cols [o|i|f|g]: s~ = tanh(pre/2) = 2*sig(pre)-1; T = tanh(g)
                # state D = 2c: M = (s~[i,f]+1)*[T|D]; D' = M2/2 + M1;
                # tch = tanh(D/2) = tanh(c); h~ = (s~o+1)*tch = 2h
                nc.scalar.activation(sigD[:, 0:32], psr[:], AF.Tanh,
                                     scale=0.5)
                nc.vector.scalar_tensor_tensor(
                    out=Mt[:], in0=sigD[:, 8:24], scalar=1.0,
                    in1=sigD[:, 24:40], op0=OP.add, op1=OP.mult)
                nc.vector.scalar_tensor_tensor(
                    out=sigD[:, 32:40], in0=Mt[:, 8:16], scalar=0.5,
                    in1=Mt[:, 0:8], op0=OP.mult, op1=OP.add)
                tch = rp.tile([128, 8], F32, tag="tch")
                nc.scalar.activation(tch[:], sigD[:, 32:40], AF.Tanh,
                                     scale=0.5)
                nc.vector.scalar_tensor_tensor(
                    out=hseq[:, 8 * t:8 * t + 8], in0=sigD[:, 0:8],
                    scalar=1.0, in1=tch[:], op0=OP.add, op1=OP.mult)

                for item in side.get(t, ()):
                    if item[0] == "tile":
                        finish_tile(item[1])
                    elif item[0] == "gemm0b":
                        gemm_range(64, 64, item[1])
                    elif item[0] in ("gemm_a", "gemm_b"):
                        t0, ntl = _CHUNKS[item[1]]
                        fn = gemm_a if item[0] == "gemm_a" else gemm_b
                        fn(128 * t0, min(128 * ntl, NT - 128 * t0), item[2])
                    elif item[0] == "probdma":
                        nc.sync.dma_start(
                            prob[:].rearrange("(t p) -> p t", p=128)[:, 0:12],
                            prob_sb[:, 0:12])
                    else:
                        out_tile(item[1])

            for t in late_out:
                out_tile(t)

            nc.sync.dma_start(
                prob[:].rearrange("(t p) -> p t", p=128)[:, 12:13],
                prob_sb[:, 12:13])

    nc.compile()
    return nc


_CACHED = None


def _get_kernel():
    global _CACHED
    if _CACHED is None:
        _CACHED = build_kernel()
    return _CACHED


def _prep_shared(pro_embed, W_ih, W_hh, b_ih, b_hh, W_out, b_out):
    wx_h = np.empty((128, 1024), np.float32)
    whh_h = np.empty((128, 512), np.float32)
    bias_h = np.empty((128, 4), np.float32)
    for j, g in enumerate(_GATE_SRC):
        blk = slice(g * 128, (g + 1) * 128)
        scx = 2.0 if j == 3 else 1.0   # g-gate preact x2: tanh((2g)/2)=tanh(g)
        sch = 1.0 if j == 3 else 0.5   # /2 for h~ = 2h feedback
        wx_h[:, j * 128:(j + 1) * 128] = scx * W_ih[blk, 0:128].T
        wx_h[:, 512 + j * 128:512 + (j + 1) * 128] = scx * W_ih[blk, 128:256].T
        whh_h[:, j * 128:(j + 1) * 128] = sch * W_hh[blk, :].T
        bias_h[:, j] = scx * (b_ih[blk] + b_hh[blk])
    wb_h = np.empty((PRO_NUM, WB), np.float32)
    wb_h[:, :H] = 0.5 * W_out
    wb_h[:, H] = 0.5 * b_out
    import ml_dtypes
    return dict(
        emb=np.ascontiguousarray(pro_embed, np.float32),
        wx=np.ascontiguousarray(wx_h).astype(ml_dtypes.bfloat16),
        whh=np.ascontiguousarray(whh_h).astype(ml_dtypes.bfloat16),
        bsum=np.ascontiguousarray(bias_h),
        wb=wb_h,
    )


def kernel(X, y, pro_embed, W_ih, W_hh, b_ih, b_hh, W_out, b_out, _trace=False,
           **_):
    X = np.asarray(X, np.int32)
    y = np.asarray(y, np.int32)
    shared = _prep_shared(np.asarray(pro_embed, np.float32),
                          np.asarray(W_ih, np.float32),
                          np.asarray(W_hh, np.float32),
                          np.asarray(b_ih, np.float32),
                          np.asarray(b_hh, np.float32),
                          np.asarray(W_out, np.float32),
                          np.asarray(b_out, np.float32))
    XT = X.T  # [200, 64]
    YT = y.T
    in_maps = []
    for c in range(N_CORES):
        cols = slice(c * BS, (c + 1) * BS)
        xtp = np.zeros((209, BS), np.int32)
        xtp[:S] = XT[:, cols]
        ytp = np.zeros((208, BS), np.int32)
        ytp[:S] = YT[:, cols]
        in_maps.append(dict(xt=xtp, yt=ytp, **shared))

    nc = _get_kernel()
    res = run_bass_kernel_spmd(nc, in_maps, core_ids=list(range(N_CORES)),
                               trace=_trace)
    out = np.empty((B, S - 1), np.float32)
    for c in range(N_CORES):
        flat = res.results[c]["prob"][:NOUT].reshape(S - 1, BS)
        out[c * BS:(c + 1) * BS, :] = flat.T
    if _trace:
        return out, res
    return out

